# revision 21
# baseline (speedup 1.0000x reference)
"""Trainium2 Bass kernel for nn_NodeAttDiff (segment-reduce node attention).

Math (reference):
    e1, e2 = out_gnn[:N], out_gnn[N:]          # N = 200000, D = 256
    diff   = e1 - e2
    h      = relu([e1 e2 diff] @ W1 + b1)      # folded: e1@WA + e2@WB, WA=W1a+W1c, WB=W1b-W1c
    raw    = (h @ W2 + b2)[:, 0]
    att    = segment_softmax(raw, batch)       # 512 contiguous segments (batch sorted)
    out    = segment_sum(att[:,None] * diff)   # [512, 256]

Device strategy (8 cores, graph-partitioned data parallel):
    - 64-ish graphs / core; each core gets its contiguous node slice (padded to
      a common capacity, pad nodes carry out-of-range segment id -> dropped).
    - Softmax max-subtraction is skipped (raw is O(5); exp is safe in fp32) and
      normalization is algebraic:  out_g = (sum_n w_n diff_n) / (sum_n w_n),
      w_n = exp(raw_n + b2)  -- no per-node att materialization.
    - Host pre-transposes e1/e2 to feature-major [2,128,cap] fp16 AND diff to
      node-major [128, nblocks, 258] fp16 (cols 256:258 = 1.0 -> the exp-sum
      denominator rides along as two extra columns of the segment matmul).
      Shipping diff node-major removes all on-device PE transposes.
    - Per group of 2x512-node tiles (weight chunks stream against both tiles):
        z^T   = WA.T @ e1T + WB.T @ e2T                  (8 matmuls/tile, PSUM)
        h^T   = relu(z^T + b1)                           (ACT, PSUM->SBUF)
        raw   = W2.T @ h^T                               (2 matmuls -> [2,512])
        ew    = exp(raw + b2)                            (ACT -> SBUF row)
        ewT   = I4-trick transpose of ew -> [128,4]      (4 tiny matmuls)
        Sw    = (iota == seg_id) * ewT                   (Pool+DVE tensor_scalar x4)
        seg  += Sw.T @ dn_block                          (PSUM accumulate, whole core)
      raw/ewT pairs of the two tiles share one PSUM bank each.
    - All constants ride in 2 DMAs (one fp16 blob + one fp32 blob) and the
      first group's tensors are fetched before them; ~32 dummy matmuls warm
      the PE DVFS p-state while the first DMAs land.
    - Tail: out = seg[:,0:256] * recip(max(seg[:,256], eps)), DMA out [gw,256].
"""

import os
import numpy as np

NUM_GRAPHS = 512
N_CORES = 8
D = 256
TILE_N = 512  # nodes per tile


_CACHE = {}


def _build_program(cap: int, gw: int):
    """Build + compile the SPMD Bass program; `cap` nodes and a `gw`-graph
    window per core."""
    if (cap, gw) in _CACHE:
        return _CACHE[(cap, gw)]

    from contextlib import ExitStack
    import concourse.bass as bass
    import concourse.tile as tile
    import concourse.bacc as bacc
    import concourse.mybir as mybir

    f32 = mybir.dt.float32
    f16 = mybir.dt.float16
    AF = mybir.ActivationFunctionType
    ALU = mybir.AluOpType

    assert cap % (2 * TILE_N) == 0
    n_tiles = cap // TILE_N
    n_groups = n_tiles // 2
    nblocks = cap // 128

    # fp16 const blob column offsets: wa, wb, w2, i4, iota
    WA_OFF = 0            # [k, m, n] flat k*256+m*128+n, 512 cols
    WB_OFF = 512
    W2_OFF = 1024         # [m, c] flat m*2+c, 4 cols
    I4_OFF = 1028         # 16 cols (partition 0 only)
    IOTA_OFF = 1044       # gw cols
    NC16 = IOTA_OFF + gw
    # fp32 blob: b1 (2 cols), b2 (1 col), bm (nblocks cols)
    NC32 = 3 + nblocks

    nc = bacc.Bacc("TRN2", target_bir_lowering=False, debug=False,
                   num_devices=N_CORES)

    e1t_d = nc.dram_tensor("e1t", [2, 128, cap], f16, kind="ExternalInput").ap()
    e2t_d = nc.dram_tensor("e2t", [2, 128, cap], f16, kind="ExternalInput").ap()
    dn_d = nc.dram_tensor("dn", [128, nblocks, D + 2], f16,
                          kind="ExternalInput").ap()
    cst_d = nc.dram_tensor("cst", [128, NC16], f16, kind="ExternalInput").ap()
    bf_d = nc.dram_tensor("bf", [128, NC32], f32, kind="ExternalInput").ap()
    out_d = nc.dram_tensor("out", [gw, D], f32, kind="ExternalOutput").ap()

    with tile.TileContext(nc) as tc:
        with ExitStack() as ctx:
            consts = ctx.enter_context(tc.tile_pool(name="consts", bufs=1))
            epool = ctx.enter_context(tc.tile_pool(name="epool", bufs=6))
            dnpool = ctx.enter_context(tc.tile_pool(name="dnpool", bufs=4))
            hpool = ctx.enter_context(tc.tile_pool(name="hpool", bufs=4))
            spool = ctx.enter_context(tc.tile_pool(name="spool", bufs=6))
            zpool = ctx.enter_context(
                tc.tile_pool(name="zpool", bufs=4, space=bass.MemorySpace.PSUM))
            rawpool = ctx.enter_context(
                tc.tile_pool(name="rawpool", bufs=1, space=bass.MemorySpace.PSUM))
            etpool = ctx.enter_context(
                tc.tile_pool(name="etpool", bufs=1, space=bass.MemorySpace.PSUM))
            segpool = ctx.enter_context(
                tc.tile_pool(name="segpool", bufs=1, space=bass.MemorySpace.PSUM))

            # ---- group input fetch (k-split DMAs; first chunks preempt
            # the consts so the MLP can start as early as possible) ----
            fetched = {}

            def fetch(g, tail=True):
                e1 = epool.tile([128, 2, 2 * TILE_N], f16, tag="e1")
                e2 = epool.tile([128, 2, 2 * TILE_N], f16, tag="e2")
                dn = dnpool.tile([128, 8, D + 2], f16, tag="dn")
                gsl = bass.ts(g, 2 * TILE_N)
                nc.sync.dma_start(e1[:, 0, :], e1t_d[0, :, gsl])
                nc.sync.dma_start(e2[:, 0, :], e2t_d[0, :, gsl])
                fetched[g] = (e1, e2, dn)
                if tail:
                    fetch_tail(g)

            def fetch_tail(g):
                e1, e2, dn = fetched[g]
                gsl = bass.ts(g, 2 * TILE_N)
                nc.sync.dma_start(e1[:, 1, :], e1t_d[1, :, gsl])
                nc.sync.dma_start(e2[:, 1, :], e2t_d[1, :, gsl])
                nc.sync.dma_start(dn[:, 0:4, :], dn_d[:, bass.ts(2 * g, 4), :])
                nc.sync.dma_start(dn[:, 4:8, :], dn_d[:, bass.ts(2 * g + 1, 4), :])

            # group 0, finest granularity: the first MLP matmuls need
            # e1/e2 k=0 tile-0 halves plus the k=0 weight chunks only
            e1_0 = epool.tile([128, 2, 2 * TILE_N], f16, tag="e1")
            e2_0 = epool.tile([128, 2, 2 * TILE_N], f16, tag="e2")
            dn_0 = dnpool.tile([128, 8, D + 2], f16, tag="dn")
            fetched[0] = (e1_0, e2_0, dn_0)
            cst = consts.tile([128, NC16], f16, tag="cst")
            bf = consts.tile([128, NC32], f32, tag="bf")
            hs = bass.ts(0, TILE_N)
            nc.sync.dma_start(e1_0[:, 0, 0:TILE_N], e1t_d[0, :, hs])
            nc.sync.dma_start(e2_0[:, 0, 0:TILE_N], e2t_d[0, :, hs])
            nc.sync.dma_start(cst[:, 0:256], cst_d[:, 0:256])        # wa k0
            nc.sync.dma_start(cst[:, 512:768], cst_d[:, 512:768])    # wb k0
            nc.sync.dma_start(bf[:], bf_d[:])
            nc.sync.dma_start(e1_0[:, 0, TILE_N:], e1t_d[0, :, bass.ts(1, TILE_N)])
            nc.sync.dma_start(e2_0[:, 0, TILE_N:], e2t_d[0, :, bass.ts(1, TILE_N)])
            nc.sync.dma_start(cst[:, 256:512], cst_d[:, 256:512])    # wa k1
            nc.sync.dma_start(cst[:, 768:1024], cst_d[:, 768:1024])  # wb k1
            nc.sync.dma_start(cst[:, 1024:], cst_d[:, 1024:])        # w2/i4/iota
            fetch_tail(0)
            if 1 < n_groups:
                fetch(1)

            def wa_ap(k, m):
                off = WA_OFF + k * 256 + m * 128
                return cst[:, off:off + 128]

            def wb_ap(k, m):
                off = WB_OFF + k * 256 + m * 128
                return cst[:, off:off + 128]

            def w2_ap(m):
                off = W2_OFF + m * 2
                return cst[:, off:off + 2]

            def i4_ap(b):
                off = I4_OFF + 4 * b
                return cst[0:1, off:off + 4]

            iota = cst[:, IOTA_OFF:IOTA_OFF + gw]

            def bm_ap(col):
                off = 3 + col
                return bf[:, off:off + 1]

            b1_ap = [bf[:, 0:1], bf[:, 1:2]]
            b2_ap = bf[0:1, 2:3]

            # ---- PE p-state warmup while first DMAs land ----
            warm = consts.tile([128, 64], f16, tag="warm")
            nc.gpsimd.memset(warm[:], 0.0)
            wps = rawpool.tile([64, 48], f32, tag="wps")
            for _ in range(32):
                nc.tensor.matmul(wps[:], warm[:, 0:64], warm[:, 0:48],
                                 start=True, stop=True)

            # seg rhs layout: [diff(256) | ones(2)] -> out cols 0:256 values,
            # 256:258 exp-sums
            seg = segpool.tile([gw, D + 2], f32, tag="seg")

            def seg_mm(sw_f, dn_f, ti_f, t_f):
                for bb in range(4):
                    nc.tensor.matmul(seg[:], sw_f[:, bb, :],
                                     dn_f[:, 4 * ti_f + bb, :],
                                     start=(t_f == 0 and bb == 0),
                                     stop=(t_f == n_tiles - 1 and bb == 3),
                                     skip_group_check=True)

            pending = []
            for g in range(n_groups):
                if g + 2 < n_groups:
                    fetch(g + 2)
                e1, e2, dn = fetched.pop(g)

                # z^T [128, 512] per (tile, fo-chunk); each weight chunk is
                # loaded once and streamed against both tiles of the group
                zc = [[None, None], [None, None]]
                for ti in range(2):
                    for m in range(2):
                        zc[ti][m] = zpool.tile([128, TILE_N], f32, tag="zr",
                                               name=f"z_{g}_{ti}_{m}")
                        for wi, (wsel, esrc, k) in enumerate(
                                [(0, e1, 0), (1, e2, 0), (0, e1, 1), (1, e2, 1)]):
                            wmat = wa_ap(k, m) if wsel == 0 else wb_ap(k, m)
                            nc.tensor.matmul(
                                zc[ti][m][:], wmat,
                                esrc[:, k, bass.ts(ti, TILE_N)],
                                start=(wi == 0), stop=(wi == 3))

                # shared-bank PSUM pairs for raw ([2,512] rows at 32ti) and
                # ewT ([128,4] cols at 4ti)
                rawp = rawpool.tile([64, TILE_N], f32, tag="raw")
                ewt_ps = etpool.tile([128, 8], f32, tag="ewt_ps")

                def flush_one():
                    if not pending:
                        return
                    seg_mm(*pending.pop(0))

                last_g = (g == n_groups - 1)

                # h^T = relu(z + b1)  (ACT, PSUM -> SBUF)
                hs_ = []
                for ti in range(2):
                    h = hpool.tile([128, 2, TILE_N], f16, tag="h")
                    for m in range(2):
                        nc.scalar.activation(h[:, m, :], zc[ti][m][:], AF.Relu,
                                             bias=b1_ap[m], scale=1.0)
                    hs_.append(h)

                # raw = W2.T @ h -> rows 32ti:32ti+2 (first row real); both
                # tiles back to back in the shared bank
                for ti in range(2):
                    nc.tensor.matmul(rawp[32 * ti:32 * ti + 2, :], w2_ap(0),
                                     hs_[ti][:, 0, :],
                                     start=True, stop=False,
                                     skip_group_check=True)
                    nc.tensor.matmul(rawp[32 * ti:32 * ti + 2, :], w2_ap(1),
                                     hs_[ti][:, 1, :],
                                     start=False, stop=True,
                                     skip_group_check=True)
                if last_g:
                    flush_one()

                # ew = exp(raw + b2) -> SBUF rows [1, 512]
                ews = []
                for ti in range(2):
                    ew = spool.tile([1, TILE_N], f16, tag="ew")
                    nc.scalar.activation(ew[:], rawp[32 * ti:32 * ti + 1, :],
                                         AF.Exp, bias=b2_ap, scale=1.0)
                    ews.append(ew)

                # ewT [128, 4] at cols 4ti..: outer products with I4 rows,
                # all 8 back to back in the shared bank
                for ti in range(2):
                    for b in range(4):
                        nc.tensor.matmul(ewt_ps[:, 4 * ti:4 * ti + 4],
                                         ews[ti][:, bass.ts(b, 128)],
                                         i4_ap(b),
                                         start=(ti == 0 and b == 0),
                                         stop=(ti == 1 and b == 3),
                                         skip_group_check=True)
                if last_g:
                    flush_one()

                for ti in range(2):
                    t = 2 * g + ti
                    ewt = spool.tile([128, 4], f32, tag="ewt")
                    nc.vector.tensor_copy(ewt[:], ewt_ps[:, 4 * ti:4 * ti + 4])

                    # Sw[:, b, :] = (iota == bm_col) * ewt_col  (Pool + DVE)
                    sw = spool.tile([128, 4, gw], f16, tag="sw")
                    for b in range(4):
                        eng = nc.gpsimd if b == 0 else nc.vector
                        eng.tensor_scalar(
                            sw[:, b, :], iota, bm_ap(4 * t + b),
                            ewt[:, b:b + 1], op0=ALU.is_equal, op1=ALU.mult)

                    pending.append((sw, dn, ti, t))
                    if last_g:
                        flush_one()

                # segment accumulate, deferred by a full group so the Sw
                # chain has ~2 tiles of slack before the PE needs its output
                if not last_g:
                    ready = [p for p in pending if p[3] < 2 * g]
                    pending = [p for p in pending if p[3] >= 2 * g]
                    for p in ready:
                        seg_mm(*p)

            while pending:
                seg_mm(*pending.pop(0))

            # tail: out = seg[:, 0:256] / max(seg[:, 256], eps)

            ssum = spool.tile([gw, 1], f32, tag="ssum")
            nc.vector.tensor_scalar_max(ssum[:], seg[:, D:D + 1], 1e-30)
            rec = spool.tile([gw, 1], f32, tag="rec")
            nc.vector.reciprocal(rec[:], ssum[:])
            ot = spool.tile([gw, D], f32, tag="ot")
            nc.vector.tensor_scalar_mul(ot[:], seg[:, 0:D], rec[:])
            hgw = gw // 2
            nc.sync.dma_start(out_d[0:hgw, :], ot[0:hgw, :])
            nc.sync.dma_start(out_d[hgw:gw, :], ot[hgw:gw, :])

    nc.compile()
    _CACHE[(cap, gw)] = nc
    return nc


def _prepare(out_gnn, batch_input, W1, b1, W2, b2):
    out_gnn = np.asarray(out_gnn, dtype=np.float32)
    batch = np.asarray(batch_input, dtype=np.int64)
    W1 = np.asarray(W1, dtype=np.float32)
    b1 = np.asarray(b1, dtype=np.float32)
    W2 = np.asarray(W2, dtype=np.float32)
    b2 = np.asarray(b2, dtype=np.float32)

    half = out_gnn.shape[0] // 2
    batch = batch[:half]
    e1_all, e2_all = out_gnn[:half], out_gnn[half:]

    # Node-balanced, graph-aligned contiguous cuts. Core c handles graphs
    # [gcut[c], gcut[c+1]) and the matching contiguous node range.
    counts = np.bincount(batch, minlength=NUM_GRAPHS)
    ccum = np.concatenate([[0], np.cumsum(counts)])  # node offset per graph
    g_used = int(np.max(np.nonzero(counts)[0])) + 1 if counts.any() else 1
    gcut = np.zeros(N_CORES + 1, dtype=np.int64)
    gcut[N_CORES] = g_used
    for c in range(1, N_CORES):
        g = int(np.searchsorted(ccum, ccum[g_used] * c / N_CORES, side="left"))
        gcut[c] = min(max(g, gcut[c - 1]), g_used)
    spans = gcut[1:] - gcut[:-1]
    if spans.max() > 128:
        gcut = np.round(np.linspace(0, g_used, N_CORES + 1)).astype(np.int64)
        spans = gcut[1:] - gcut[:-1]
        if spans.max() > 128:
            raise ValueError(f"graph window {spans.max()} > 128 unsupported")

    nbounds = ccum[gcut]  # node boundaries per core
    gw = int(max(2, ((spans.max() + 1) // 2) * 2))
    max_n = int((nbounds[1:] - nbounds[:-1]).max())
    grp = 2 * TILE_N
    cap = max(grp, ((max_n + grp - 1) // grp) * grp)
    nblocks = cap // 128

    nc = _build_program(cap, gw)

    # host-folded MLP weights (fp64 for exactness)
    W1a = W1[0:D].astype(np.float64)
    W1b = W1[D:2 * D].astype(np.float64)
    W1c = W1[2 * D:3 * D].astype(np.float64)
    WA = (W1a + W1c).astype(np.float32)
    WB = (W1b - W1c).astype(np.float32)

    def chunkw(w):  # [256,256] -> [k*256+m*128+n] flat fp16 cols on 128 rows
        return np.ascontiguousarray(
            w.astype(np.float16).reshape(2, 128, 2, 128).transpose(1, 0, 2, 3)
            .reshape(128, 512))

    # fp16 const blob
    IOTA_OFF = 1044
    NC16 = IOTA_OFF + gw
    cst_common = np.zeros((128, NC16), dtype=np.float16)
    cst_common[:, 0:512] = chunkw(WA)
    cst_common[:, 512:1024] = chunkw(WB)
    cst_common[:, 1024:1028] = np.concatenate(
        [W2.astype(np.float16).reshape(2, 128, 1).transpose(1, 0, 2),
         np.zeros((128, 2, 1), np.float16)], axis=2).reshape(128, 4)
    cst_common[0, 1028:1044] = np.eye(4, dtype=np.float16).reshape(16)
    cst_common[:, IOTA_OFF:IOTA_OFF + gw] = np.arange(gw, dtype=np.float16)

    bf_common = np.zeros((128, 3 + nblocks), dtype=np.float32)
    bf_common[:, 0] = b1[0:128]
    bf_common[:, 1] = b1[128:256]
    bf_common[0, 2] = b2[0]

    in_maps = []
    for c in range(N_CORES):
        s, e = int(nbounds[c]), int(nbounds[c + 1])
        n_c = e - s
        e1t = np.zeros((2, 128, cap), dtype=np.float16)
        e2t = np.zeros((2, 128, cap), dtype=np.float16)
        e1t[:, :, :n_c] = e1_all[s:e].astype(np.float16).T.reshape(2, 128, n_c)
        e2t[:, :, :n_c] = e2_all[s:e].astype(np.float16).T.reshape(2, 128, n_c)
        # node-major diff blob [128, nblocks, 258]; cols 256:258 = 1.0
        dnb = np.zeros((nblocks, 128, D + 2), dtype=np.float16)
        dpad = np.zeros((cap, D), dtype=np.float16)
        dpad[:n_c] = (e1_all[s:e] - e2_all[s:e]).astype(np.float16)
        dnb[:, :, 0:D] = dpad.reshape(nblocks, 128, D)
        dnb[:, :, D:D + 2] = 1.0
        dnb = np.ascontiguousarray(dnb.transpose(1, 0, 2))
        bmv = np.full(cap, 999.0, dtype=np.float32)
        bmv[:n_c] = (batch[s:e] - gcut[c]).astype(np.float32)
        bf = bf_common.copy()
        bf[:, 3:3 + nblocks] = bmv.reshape(nblocks, 128).T
        in_maps.append({
            "e1t": e1t, "e2t": e2t, "dn": dnb, "cst": cst_common, "bf": bf,
        })
    return nc, in_maps, gcut


def _enable_ldw_opt():
    """Re-enable the compiler's weight-load optimization (off by default in
    this container's flag set); harmless no-op if the flag isn't present."""
    try:
        from concourse.compiler_utils import get_compiler_flags, set_compiler_flags
        flags = [f.replace("--enable-ldw-opt=false", "--enable-ldw-opt=true")
                 for f in get_compiler_flags()]
        set_compiler_flags(flags)
    except Exception:
        pass


def kernel(out_gnn, batch_input, W1, b1, W2, b2):
    import concourse.bass_utils as bass_utils

    _enable_ldw_opt()
    nc, in_maps, gcut = _prepare(out_gnn, batch_input, W1, b1, W2, b2)

    trace_dir = os.environ.get("NODEATT_TRACE_DIR")
    kw = {}
    if trace_dir:
        kw = {"trace": True, "tmpdir": trace_dir}
    res = bass_utils.run_bass_kernel_spmd(
        nc, in_maps, core_ids=list(range(N_CORES)), **kw)
    if trace_dir:
        kernel.last_exec_time_ns = res.exec_time_ns
        kernel.last_results = res

    out = np.zeros((NUM_GRAPHS, D), dtype=np.float32)
    for c in range(N_CORES):
        span = int(gcut[c + 1] - gcut[c])
        if span > 0:
            out[gcut[c]:gcut[c + 1]] = res.results[c]["out"][:span]
    return out


# revision 30
# speedup vs baseline: 1.0281x; 1.0281x over previous
"""Trainium2 Bass kernel for nn_NodeAttDiff (segment-reduce node attention).

Math (reference):
    e1, e2 = out_gnn[:N], out_gnn[N:]          # N = 200000, D = 256
    diff   = e1 - e2
    h      = relu([e1 e2 diff] @ W1 + b1)      # folded: e1@WA + e2@WB, WA=W1a+W1c, WB=W1b-W1c
    raw    = (h @ W2 + b2)[:, 0]
    att    = segment_softmax(raw, batch)       # 512 contiguous segments (batch sorted)
    out    = segment_sum(att[:,None] * diff)   # [512, 256]

Device strategy (8 cores, graph-partitioned data parallel):
    - 64-ish graphs / core; each core gets its contiguous node slice (padded to
      a common capacity, pad nodes carry out-of-range segment id -> dropped).
    - Softmax max-subtraction is skipped (raw is O(5); exp is safe in fp32) and
      normalization is algebraic:  out_g = (sum_n w_n diff_n) / (sum_n w_n),
      w_n = exp(raw_n + b2)  -- no per-node att materialization.
    - Host pre-transposes e1/e2 to feature-major [2,128,cap] fp16 AND diff to
      node-major [128, nblocks, 258] fp16 (cols 256:258 = 1.0 -> the exp-sum
      denominator rides along as two extra columns of the segment matmul).
      Shipping diff node-major removes all on-device PE transposes.
    - Per group of 2x512-node tiles (weight chunks stream against both tiles):
        z^T   = WA.T @ e1T + WB.T @ e2T                  (8 matmuls/tile, PSUM)
        h^T   = relu(z^T + b1)                           (ACT, PSUM->SBUF)
        raw   = W2.T @ h^T                               (2 matmuls -> [2,512])
        ew    = exp(raw + b2)                            (ACT -> SBUF row)
        ewT   = I4-trick transpose of ew -> [128,4]      (4 tiny matmuls)
        Sw    = (iota == seg_id) * ewT                   (Pool+DVE tensor_scalar x4)
        seg  += Sw.T @ dn_block                          (PSUM accumulate, whole core)
      raw/ewT pairs of the two tiles share one PSUM bank each.
    - All constants ride in a few split DMAs (fp16 blob + fp32 blob) behind
      the first group's e-tensors; 44 dummy matmuls keep the PE busy while
      the first DMAs land so the DVFS p-state ramps before real work.
    - dn fetches trail e-fetches by a group (seg use is deferred a group),
      keeping head DMA bandwidth on the MLP-critical e-tiles.
    - Tail: out = seg[:,0:256] * recip(max(seg[:,256], eps)), DMA out [gw,256].
"""

import os
import numpy as np

NUM_GRAPHS = 512
N_CORES = 8
D = 256
TILE_N = 512  # nodes per tile


_CACHE = {}


def _build_program(cap: int, gw: int):
    """Build + compile the SPMD Bass program; `cap` nodes and a `gw`-graph
    window per core."""
    if (cap, gw) in _CACHE:
        return _CACHE[(cap, gw)]

    from contextlib import ExitStack
    import concourse.bass as bass
    import concourse.tile as tile
    import concourse.bacc as bacc
    import concourse.mybir as mybir

    f32 = mybir.dt.float32
    f16 = mybir.dt.float16
    AF = mybir.ActivationFunctionType
    ALU = mybir.AluOpType

    assert cap % (2 * TILE_N) == 0
    n_tiles = cap // TILE_N
    n_groups = n_tiles // 2
    nblocks = cap // 128

    # fp16 const blob: per-k weight banks [waKm0 waKm1 wbKm0 wbKm1], k=0,1
    W2_OFF = 1024         # [m, c] flat m*2+c, 4 cols
    I4_OFF = 1028         # 16 cols (partition 0 only)
    IOTA_OFF = 1044       # gw cols
    NC16 = IOTA_OFF + gw
    # fp32 blob: b1 (2 cols), b2 (1 col), bm (nblocks cols)
    NC32 = 3 + nblocks

    nc = bacc.Bacc("TRN2", target_bir_lowering=False, debug=False,
                   num_devices=N_CORES)

    e1t_d = nc.dram_tensor("e1t", [2, 128, cap], f16, kind="ExternalInput").ap()
    e2t_d = nc.dram_tensor("e2t", [2, 128, cap], f16, kind="ExternalInput").ap()
    dn_d = nc.dram_tensor("dn", [128, nblocks, D + 2], f16,
                          kind="ExternalInput").ap()
    cst_d = nc.dram_tensor("cst", [128, NC16], f16, kind="ExternalInput").ap()
    bf_d = nc.dram_tensor("bf", [128, NC32], f32, kind="ExternalInput").ap()
    out_d = nc.dram_tensor("out", [gw, D], f32, kind="ExternalOutput").ap()

    with tile.TileContext(nc) as tc:
        with ExitStack() as ctx:
            consts = ctx.enter_context(tc.tile_pool(name="consts", bufs=1))
            epool = ctx.enter_context(tc.tile_pool(name="epool", bufs=6))
            dnpool = ctx.enter_context(tc.tile_pool(name="dnpool", bufs=4))
            hpool = ctx.enter_context(tc.tile_pool(name="hpool", bufs=4))
            spool = ctx.enter_context(tc.tile_pool(name="spool", bufs=8))
            zpool = ctx.enter_context(
                tc.tile_pool(name="zpool", bufs=4, space=bass.MemorySpace.PSUM))
            rawpool = ctx.enter_context(
                tc.tile_pool(name="rawpool", bufs=1, space=bass.MemorySpace.PSUM))
            etpool = ctx.enter_context(
                tc.tile_pool(name="etpool", bufs=1, space=bass.MemorySpace.PSUM))
            segpool = ctx.enter_context(
                tc.tile_pool(name="segpool", bufs=1, space=bass.MemorySpace.PSUM))

            # ---- group input fetch (k-split DMAs; first chunks preempt
            # the consts so the MLP can start as early as possible) ----
            fetched = {}

            def fetch(g, tail=True):
                e1 = epool.tile([128, 2, 2 * TILE_N], f16, tag="e1")
                e2 = epool.tile([128, 2, 2 * TILE_N], f16, tag="e2")
                dn = dnpool.tile([128, 8, D + 2], f16, tag="dn")
                gsl = bass.ts(g, 2 * TILE_N)
                nc.sync.dma_start(e1[:, 0, :], e1t_d[0, :, gsl])
                nc.sync.dma_start(e2[:, 0, :], e2t_d[0, :, gsl])
                fetched[g] = (e1, e2, dn)
                if tail:
                    fetch_tail(g)

            def fetch_tail(g):
                e1, e2, dn = fetched[g]
                gsl = bass.ts(g, 2 * TILE_N)
                nc.sync.dma_start(e1[:, 1, :], e1t_d[1, :, gsl])
                nc.sync.dma_start(e2[:, 1, :], e2t_d[1, :, gsl])

            def fetch_dn(g):
                dn = fetched[g][2]
                nc.sync.dma_start(dn[:], dn_d[:, bass.ts(g, 8), :])

            # group 0, finest granularity: the first MLP matmuls need
            # e1/e2 k=0 tile-0 halves plus the k=0 weight chunks only
            e1_0 = epool.tile([128, 2, 2 * TILE_N], f16, tag="e1")
            e2_0 = epool.tile([128, 2, 2 * TILE_N], f16, tag="e2")
            dn_0 = dnpool.tile([128, 8, D + 2], f16, tag="dn")
            fetched[0] = (e1_0, e2_0, dn_0)
            cst = consts.tile([128, NC16], f16, tag="cst")
            bf = consts.tile([128, NC32], f32, tag="bf")
            # issue order follows the ti-outer MLP's consumption order:
            # tile-0 needs k0 AND k1 (data + weights) within 4 matmuls
            hs = bass.ts(0, TILE_N)
            t1 = bass.ts(1, TILE_N)
            nc.sync.dma_start(e1_0[:, 0, 0:TILE_N], e1t_d[0, :, hs])
            nc.sync.dma_start(e2_0[:, 0, 0:TILE_N], e2t_d[0, :, hs])
            nc.sync.dma_start(cst[:, 0:512], cst_d[:, 0:512])        # k0 weights
            nc.sync.dma_start(e1_0[:, 1, 0:TILE_N], e1t_d[1, :, hs])
            nc.sync.dma_start(e2_0[:, 1, 0:TILE_N], e2t_d[1, :, hs])
            nc.sync.dma_start(cst[:, 512:1024], cst_d[:, 512:1024])  # k1 weights
            nc.sync.dma_start(bf[:], bf_d[:])
            nc.sync.dma_start(e1_0[:, 0, TILE_N:], e1t_d[0, :, t1])
            nc.sync.dma_start(e2_0[:, 0, TILE_N:], e2t_d[0, :, t1])
            nc.sync.dma_start(e1_0[:, 1, TILE_N:], e1t_d[1, :, t1])
            nc.sync.dma_start(e2_0[:, 1, TILE_N:], e2t_d[1, :, t1])
            nc.sync.dma_start(cst[:, 1024:], cst_d[:, 1024:])        # w2/i4/iota
            if 1 < n_groups:
                fetch(1)

            def wa_ap(k, m):
                off = 512 * k + 128 * m
                return cst[:, off:off + 128]

            def wb_ap(k, m):
                off = 512 * k + 256 + 128 * m
                return cst[:, off:off + 128]

            def w2_ap(m):
                off = W2_OFF + m * 2
                return cst[:, off:off + 2]

            def i4_ap(b):
                off = I4_OFF + 4 * b
                return cst[0:1, off:off + 4]

            iota = cst[:, IOTA_OFF:IOTA_OFF + gw]

            def bm_ap(col):
                off = 3 + col
                return bf[:, off:off + 1]

            b1_ap = [bf[:, 0:1], bf[:, 1:2]]
            b2_ap = bf[0:1, 2:3]

            # ---- PE p-state warmup while first DMAs land ----
            warm = consts.tile([128, 64], f16, tag="warm")
            nc.gpsimd.memset(warm[:], 0.0)
            wps = rawpool.tile([64, 48], f32, tag="wps")
            for _ in range(44):
                nc.tensor.matmul(wps[:], warm[:, 0:64], warm[:, 0:48],
                                 start=True, stop=True)

            # seg rhs layout: [diff(256) | ones(2)] -> out cols 0:256 values,
            # 256:258 exp-sums
            seg = segpool.tile([gw, D + 2], f32, tag="seg")

            def seg_mm(sw_f, dn_f, ti_f, t_f):
                for bb in range(4):
                    nc.tensor.matmul(seg[:], sw_f[:, bb, :],
                                     dn_f[:, 4 * ti_f + bb, :],
                                     start=(t_f == 0 and bb == 0),
                                     stop=(t_f == n_tiles - 1 and bb == 3),
                                     skip_group_check=True)

            fetch_dn(0)
            pending = []
            for g in range(n_groups):
                if g + 2 < n_groups:
                    fetch(g + 2)
                if g + 1 < n_groups:
                    fetch_dn(g + 1)
                e1, e2, dn = fetched[g]
                del fetched[g]

                # z^T [128, 512] per (tile, fo-chunk); each weight chunk is
                # loaded once and streamed against both tiles of the group
                zc = [[None, None], [None, None]]
                for ti in range(2):
                    for m in range(2):
                        zc[ti][m] = zpool.tile([128, TILE_N], f32, tag="zr",
                                               name=f"z_{g}_{ti}_{m}")
                        for wi, (wsel, esrc, k) in enumerate(
                                [(0, e1, 0), (1, e2, 0), (0, e1, 1), (1, e2, 1)]):
                            wmat = wa_ap(k, m) if wsel == 0 else wb_ap(k, m)
                            nc.tensor.matmul(
                                zc[ti][m][:], wmat,
                                esrc[:, k, bass.ts(ti, TILE_N)],
                                start=(wi == 0), stop=(wi == 3))

                # shared-bank PSUM pairs for raw ([2,512] rows at 32ti) and
                # ewT ([128,4] cols at 4ti)
                rawp = rawpool.tile([64, TILE_N], f32, tag="raw")
                ewt_ps = etpool.tile([128, 8], f32, tag="ewt_ps")

                def flush_one():
                    if not pending:
                        return
                    seg_mm(*pending.pop(0))

                last_g = (g == n_groups - 1)
                fill_g = (g >= n_groups - 2)

                # h^T = relu(z + b1)  (ACT, PSUM -> SBUF)
                hs_ = []
                for ti in range(2):
                    h = hpool.tile([128, 2, TILE_N], f16, tag="h")
                    for m in range(2):
                        nc.scalar.activation(h[:, m, :], zc[ti][m][:], AF.Relu,
                                             bias=b1_ap[m], scale=1.0)
                    hs_.append(h)

                if last_g:
                    flush_one()
                    flush_one()

                # raw = W2.T @ h -> rows 32ti:32ti+2 (first row real); both
                # tiles back to back in the shared bank
                for ti in range(2):
                    nc.tensor.matmul(rawp[32 * ti:32 * ti + 2, :], w2_ap(0),
                                     hs_[ti][:, 0, :],
                                     start=True, stop=False,
                                     skip_group_check=True)
                    nc.tensor.matmul(rawp[32 * ti:32 * ti + 2, :], w2_ap(1),
                                     hs_[ti][:, 1, :],
                                     start=False, stop=True,
                                     skip_group_check=True)
                if last_g:
                    flush_one()

                # ew = exp(raw + b2) -> SBUF rows [1, 512]
                ews = []
                for ti in range(2):
                    ew = spool.tile([1, TILE_N], f16, tag="ew")
                    nc.scalar.activation(ew[:], rawp[32 * ti:32 * ti + 1, :],
                                         AF.Exp, bias=b2_ap, scale=1.0)
                    ews.append(ew)

                # ewT [128, 4] at cols 4ti..: outer products with I4 rows,
                # all 8 back to back in the shared bank
                for ti in range(2):
                    for b in range(4):
                        nc.tensor.matmul(ewt_ps[:, 4 * ti:4 * ti + 4],
                                         ews[ti][:, bass.ts(b, 128)],
                                         i4_ap(b),
                                         start=(ti == 0 and b == 0),
                                         stop=(ti == 1 and b == 3),
                                         skip_group_check=True)
                if last_g:
                    flush_one()

                for ti in range(2):
                    t = 2 * g + ti
                    ewt = spool.tile([128, 4], f32, tag="ewt")
                    nc.vector.tensor_copy(ewt[:], ewt_ps[:, 4 * ti:4 * ti + 4])

                    # Sw[:, b, :] = (iota == bm_col) * ewt_col  (Pool + DVE)
                    sw = spool.tile([128, 4, gw], f16, tag="sw")
                    for b in range(4):
                        eng = nc.gpsimd if b == 0 else nc.vector
                        eng.tensor_scalar(
                            sw[:, b, :], iota, bm_ap(4 * t + b),
                            ewt[:, b:b + 1], op0=ALU.is_equal, op1=ALU.mult)

                    pending.append((sw, dn, ti, t))

                # segment accumulate, deferred by a full group so the Sw
                # chain has ~2 tiles of slack before the PE needs its output
                # (the last two groups defer everything so the final group's
                # latency chain is covered by old seg work)
                if not fill_g:
                    ready = [p for p in pending if p[3] < 2 * g]
                    pending = [p for p in pending if p[3] >= 2 * g]
                    for p in ready:
                        seg_mm(*p)

            while pending:
                seg_mm(*pending.pop(0))

            # tail: out = seg[:, 0:256] / max(seg[:, 256], eps)

            ssum = spool.tile([gw, 1], f32, tag="ssum")
            nc.vector.tensor_scalar_max(ssum[:], seg[:, D:D + 1], 1e-30)
            rec = spool.tile([gw, 1], f32, tag="rec")
            nc.vector.reciprocal(rec[:], ssum[:])
            ot = spool.tile([gw, D], f32, tag="ot")
            nc.vector.tensor_scalar_mul(ot[:], seg[:, 0:D], rec[:])
            hgw = gw // 2
            nc.sync.dma_start(out_d[0:hgw, :], ot[0:hgw, :])
            nc.sync.dma_start(out_d[hgw:gw, :], ot[hgw:gw, :])

    nc.compile()
    _CACHE[(cap, gw)] = nc
    return nc


def _prepare(out_gnn, batch_input, W1, b1, W2, b2):
    out_gnn = np.asarray(out_gnn, dtype=np.float32)
    batch = np.asarray(batch_input, dtype=np.int64)
    W1 = np.asarray(W1, dtype=np.float32)
    b1 = np.asarray(b1, dtype=np.float32)
    W2 = np.asarray(W2, dtype=np.float32)
    b2 = np.asarray(b2, dtype=np.float32)

    half = out_gnn.shape[0] // 2
    batch = batch[:half]
    e1_all, e2_all = out_gnn[:half], out_gnn[half:]

    # Node-balanced, graph-aligned contiguous cuts. Core c handles graphs
    # [gcut[c], gcut[c+1]) and the matching contiguous node range.
    counts = np.bincount(batch, minlength=NUM_GRAPHS)
    ccum = np.concatenate([[0], np.cumsum(counts)])  # node offset per graph
    g_used = int(np.max(np.nonzero(counts)[0])) + 1 if counts.any() else 1
    gcut = np.zeros(N_CORES + 1, dtype=np.int64)
    gcut[N_CORES] = g_used
    for c in range(1, N_CORES):
        g = int(np.searchsorted(ccum, ccum[g_used] * c / N_CORES, side="left"))
        gcut[c] = min(max(g, gcut[c - 1]), g_used)
    spans = gcut[1:] - gcut[:-1]
    if spans.max() > 128:
        gcut = np.round(np.linspace(0, g_used, N_CORES + 1)).astype(np.int64)
        spans = gcut[1:] - gcut[:-1]
        if spans.max() > 128:
            raise ValueError(f"graph window {spans.max()} > 128 unsupported")

    nbounds = ccum[gcut]  # node boundaries per core
    gw = int(max(2, ((spans.max() + 1) // 2) * 2))
    max_n = int((nbounds[1:] - nbounds[:-1]).max())
    grp = 2 * TILE_N
    cap = max(grp, ((max_n + grp - 1) // grp) * grp)
    nblocks = cap // 128

    nc = _build_program(cap, gw)

    # host-folded MLP weights (fp64 for exactness)
    W1a = W1[0:D].astype(np.float64)
    W1b = W1[D:2 * D].astype(np.float64)
    W1c = W1[2 * D:3 * D].astype(np.float64)
    WA = (W1a + W1c).astype(np.float32)
    WB = (W1b - W1c).astype(np.float32)

    def chunkw(w, k):  # [256,256] -> [m*128+n] fp16 cols on 128 rows, chunk k
        return np.ascontiguousarray(
            w[k * 128:(k + 1) * 128].astype(np.float16).reshape(128, 256))

    # fp16 const blob
    IOTA_OFF = 1044
    NC16 = IOTA_OFF + gw
    cst_common = np.zeros((128, NC16), dtype=np.float16)
    for k in range(2):
        cst_common[:, 512 * k:512 * k + 256] = chunkw(WA, k)
        cst_common[:, 512 * k + 256:512 * k + 512] = chunkw(WB, k)
    cst_common[:, 1024:1028] = np.concatenate(
        [W2.astype(np.float16).reshape(2, 128, 1).transpose(1, 0, 2),
         np.zeros((128, 2, 1), np.float16)], axis=2).reshape(128, 4)
    cst_common[0, 1028:1044] = np.eye(4, dtype=np.float16).reshape(16)
    cst_common[:, IOTA_OFF:IOTA_OFF + gw] = np.arange(gw, dtype=np.float16)

    bf_common = np.zeros((128, 3 + nblocks), dtype=np.float32)
    bf_common[:, 0] = b1[0:128]
    bf_common[:, 1] = b1[128:256]
    bf_common[0, 2] = b2[0]

    in_maps = []
    for c in range(N_CORES):
        s, e = int(nbounds[c]), int(nbounds[c + 1])
        n_c = e - s
        e1t = np.zeros((2, 128, cap), dtype=np.float16)
        e2t = np.zeros((2, 128, cap), dtype=np.float16)
        e1t[:, :, :n_c] = e1_all[s:e].astype(np.float16).T.reshape(2, 128, n_c)
        e2t[:, :, :n_c] = e2_all[s:e].astype(np.float16).T.reshape(2, 128, n_c)
        # node-major diff blob [128, nblocks, 258]; cols 256:258 = 1.0
        dnb = np.zeros((nblocks, 128, D + 2), dtype=np.float16)
        dpad = np.zeros((cap, D), dtype=np.float16)
        dpad[:n_c] = (e1_all[s:e] - e2_all[s:e]).astype(np.float16)
        dnb[:, :, 0:D] = dpad.reshape(nblocks, 128, D)
        dnb[:, :, D:D + 2] = 1.0
        dnb = np.ascontiguousarray(dnb.transpose(1, 0, 2))
        bmv = np.full(cap, 999.0, dtype=np.float32)
        bmv[:n_c] = (batch[s:e] - gcut[c]).astype(np.float32)
        bf = bf_common.copy()
        bf[:, 3:3 + nblocks] = bmv.reshape(nblocks, 128).T
        in_maps.append({
            "e1t": e1t, "e2t": e2t, "dn": dnb, "cst": cst_common, "bf": bf,
        })
    return nc, in_maps, gcut


def _enable_ldw_opt():
    """Re-enable the compiler's weight-load optimization (off by default in
    this container's flag set); harmless no-op if the flag isn't present."""
    try:
        from concourse.compiler_utils import get_compiler_flags, set_compiler_flags
        flags = [f.replace("--enable-ldw-opt=false", "--enable-ldw-opt=true")
                 for f in get_compiler_flags()]
        set_compiler_flags(flags)
    except Exception:
        pass


def kernel(out_gnn, batch_input, W1, b1, W2, b2):
    import concourse.bass_utils as bass_utils

    _enable_ldw_opt()
    nc, in_maps, gcut = _prepare(out_gnn, batch_input, W1, b1, W2, b2)

    trace_dir = os.environ.get("NODEATT_TRACE_DIR")
    kw = {}
    if trace_dir:
        kw = {"trace": True, "tmpdir": trace_dir}
    res = bass_utils.run_bass_kernel_spmd(
        nc, in_maps, core_ids=list(range(N_CORES)), **kw)
    if trace_dir:
        kernel.last_exec_time_ns = res.exec_time_ns
        kernel.last_results = res

    out = np.zeros((NUM_GRAPHS, D), dtype=np.float32)
    for c in range(N_CORES):
        span = int(gcut[c + 1] - gcut[c])
        if span > 0:
            out[gcut[c]:gcut[c + 1]] = res.results[c]["out"][:span]
    return out


# revision 32
# speedup vs baseline: 1.0282x; 1.0001x over previous
"""Trainium2 Bass kernel for nn_NodeAttDiff (segment-reduce node attention).

Math (reference):
    e1, e2 = out_gnn[:N], out_gnn[N:]          # N = 200000, D = 256
    diff   = e1 - e2
    h      = relu([e1 e2 diff] @ W1 + b1)      # folded: e1@WA + e2@WB, WA=W1a+W1c, WB=W1b-W1c
    raw    = (h @ W2 + b2)[:, 0]
    att    = segment_softmax(raw, batch)       # 512 contiguous segments (batch sorted)
    out    = segment_sum(att[:,None] * diff)   # [512, 256]

Device strategy (8 cores, graph-partitioned data parallel):
    - 64-ish graphs / core; each core gets its contiguous node slice (padded to
      a common capacity, pad nodes carry out-of-range segment id -> dropped).
    - Softmax max-subtraction is skipped (raw is O(5); exp is safe in fp32) and
      normalization is algebraic:  out_g = (sum_n w_n diff_n) / (sum_n w_n),
      w_n = exp(raw_n + b2)  -- no per-node att materialization.
    - Host pre-transposes e1/e2 to feature-major [2,128,cap] fp16 AND diff to
      node-major [128, nblocks, 258] fp16 (cols 256:258 = 1.0 -> the exp-sum
      denominator rides along as two extra columns of the segment matmul).
      Shipping diff node-major removes all on-device PE transposes.
    - Per group of 2x512-node tiles (weight chunks stream against both tiles):
        z^T   = WA.T @ e1T + WB.T @ e2T                  (8 matmuls/tile, PSUM)
        h^T   = relu(z^T + b1)                           (ACT, PSUM->SBUF)
        raw   = W2.T @ h^T                               (2 matmuls -> [2,512])
        ew    = exp(raw + b2)                            (ACT -> SBUF row)
        ewT   = I4-trick transpose of ew -> [128,4]      (4 tiny matmuls)
        Sw    = (iota == seg_id) * ewT                   (Pool+DVE tensor_scalar x4)
        seg  += Sw.T @ dn_block                          (PSUM accumulate, whole core)
      raw/ewT pairs of the two tiles share one PSUM bank each.
    - All constants ride in a few split DMAs (fp16 blob + fp32 blob) behind
      the first group's e-tensors; 44 dummy matmuls keep the PE busy while
      the first DMAs land so the DVFS p-state ramps before real work.
    - dn fetches trail e-fetches by a group (seg use is deferred a group),
      keeping head DMA bandwidth on the MLP-critical e-tiles.
    - Tail: out = seg[:,0:256] * recip(max(seg[:,256], eps)), DMA out [gw,256].
"""

import os
import numpy as np

NUM_GRAPHS = 512
N_CORES = 8
D = 256
TILE_N = 512  # nodes per tile


_CACHE = {}


def _build_program(cap: int, gw: int, skip_blocks: int = 0):
    """Build + compile the SPMD Bass program; `cap` nodes and a `gw`-graph
    window per core. The trailing `skip_blocks` 128-node blocks are pad on
    every core and get no sw/seg work."""
    if (cap, gw, skip_blocks) in _CACHE:
        return _CACHE[(cap, gw, skip_blocks)]

    from contextlib import ExitStack
    import concourse.bass as bass
    import concourse.tile as tile
    import concourse.bacc as bacc
    import concourse.mybir as mybir

    f32 = mybir.dt.float32
    f16 = mybir.dt.float16
    AF = mybir.ActivationFunctionType
    ALU = mybir.AluOpType

    assert cap % (2 * TILE_N) == 0
    n_tiles = cap // TILE_N
    n_groups = n_tiles // 2
    nblocks = cap // 128

    # fp16 const blob: per-k weight banks [waKm0 waKm1 wbKm0 wbKm1], k=0,1
    W2_OFF = 1024         # [m, c] flat m*2+c, 4 cols
    I4_OFF = 1028         # 16 cols (partition 0 only)
    IOTA_OFF = 1044       # gw cols
    NC16 = IOTA_OFF + gw
    # fp32 blob: b1 (2 cols), b2 (1 col), bm (nblocks cols)
    NC32 = 3 + nblocks

    nc = bacc.Bacc("TRN2", target_bir_lowering=False, debug=False,
                   num_devices=N_CORES)

    e1t_d = nc.dram_tensor("e1t", [2, 128, cap], f16, kind="ExternalInput").ap()
    e2t_d = nc.dram_tensor("e2t", [2, 128, cap], f16, kind="ExternalInput").ap()
    dn_d = nc.dram_tensor("dn", [128, nblocks, D + 2], f16,
                          kind="ExternalInput").ap()
    cst_d = nc.dram_tensor("cst", [128, NC16], f16, kind="ExternalInput").ap()
    bf_d = nc.dram_tensor("bf", [128, NC32], f32, kind="ExternalInput").ap()
    out_d = nc.dram_tensor("out", [gw, D], f32, kind="ExternalOutput").ap()

    with tile.TileContext(nc) as tc:
        with ExitStack() as ctx:
            consts = ctx.enter_context(tc.tile_pool(name="consts", bufs=1))
            epool = ctx.enter_context(tc.tile_pool(name="epool", bufs=6))
            dnpool = ctx.enter_context(tc.tile_pool(name="dnpool", bufs=4))
            hpool = ctx.enter_context(tc.tile_pool(name="hpool", bufs=4))
            spool = ctx.enter_context(tc.tile_pool(name="spool", bufs=8))
            zpool = ctx.enter_context(
                tc.tile_pool(name="zpool", bufs=4, space=bass.MemorySpace.PSUM))
            rawpool = ctx.enter_context(
                tc.tile_pool(name="rawpool", bufs=1, space=bass.MemorySpace.PSUM))
            etpool = ctx.enter_context(
                tc.tile_pool(name="etpool", bufs=1, space=bass.MemorySpace.PSUM))
            segpool = ctx.enter_context(
                tc.tile_pool(name="segpool", bufs=1, space=bass.MemorySpace.PSUM))

            # ---- group input fetch (k-split DMAs; first chunks preempt
            # the consts so the MLP can start as early as possible) ----
            fetched = {}

            def fetch(g, tail=True):
                e1 = epool.tile([128, 2, 2 * TILE_N], f16, tag="e1")
                e2 = epool.tile([128, 2, 2 * TILE_N], f16, tag="e2")
                dn = dnpool.tile([128, 8, D + 2], f16, tag="dn")
                gsl = bass.ts(g, 2 * TILE_N)
                nc.sync.dma_start(e1[:, 0, :], e1t_d[0, :, gsl])
                nc.sync.dma_start(e2[:, 0, :], e2t_d[0, :, gsl])
                fetched[g] = (e1, e2, dn)
                if tail:
                    fetch_tail(g)

            def fetch_tail(g):
                e1, e2, dn = fetched[g]
                gsl = bass.ts(g, 2 * TILE_N)
                nc.sync.dma_start(e1[:, 1, :], e1t_d[1, :, gsl])
                nc.sync.dma_start(e2[:, 1, :], e2t_d[1, :, gsl])

            def fetch_dn(g):
                dn = fetched[g][2]
                nc.sync.dma_start(dn[:], dn_d[:, bass.ts(g, 8), :])

            # group 0, finest granularity: the first MLP matmuls need
            # e1/e2 k=0 tile-0 halves plus the k=0 weight chunks only
            e1_0 = epool.tile([128, 2, 2 * TILE_N], f16, tag="e1")
            e2_0 = epool.tile([128, 2, 2 * TILE_N], f16, tag="e2")
            dn_0 = dnpool.tile([128, 8, D + 2], f16, tag="dn")
            fetched[0] = (e1_0, e2_0, dn_0)
            cst = consts.tile([128, NC16], f16, tag="cst")
            bf = consts.tile([128, NC32], f32, tag="bf")
            # issue order follows the ti-outer MLP's consumption order:
            # tile-0 needs k0 AND k1 (data + weights) within 4 matmuls
            hs = bass.ts(0, TILE_N)
            t1 = bass.ts(1, TILE_N)
            nc.sync.dma_start(e1_0[:, 0, 0:TILE_N], e1t_d[0, :, hs])
            nc.sync.dma_start(e2_0[:, 0, 0:TILE_N], e2t_d[0, :, hs])
            nc.sync.dma_start(cst[:, 0:512], cst_d[:, 0:512])        # k0 weights
            nc.sync.dma_start(e1_0[:, 1, 0:TILE_N], e1t_d[1, :, hs])
            nc.sync.dma_start(e2_0[:, 1, 0:TILE_N], e2t_d[1, :, hs])
            nc.sync.dma_start(cst[:, 512:1024], cst_d[:, 512:1024])  # k1 weights
            nc.sync.dma_start(bf[:], bf_d[:])
            nc.sync.dma_start(e1_0[:, 0, TILE_N:], e1t_d[0, :, t1])
            nc.sync.dma_start(e2_0[:, 0, TILE_N:], e2t_d[0, :, t1])
            nc.sync.dma_start(e1_0[:, 1, TILE_N:], e1t_d[1, :, t1])
            nc.sync.dma_start(e2_0[:, 1, TILE_N:], e2t_d[1, :, t1])
            nc.sync.dma_start(cst[:, 1024:], cst_d[:, 1024:])        # w2/i4/iota
            if 1 < n_groups:
                fetch(1)

            def wa_ap(k, m):
                off = 512 * k + 128 * m
                return cst[:, off:off + 128]

            def wb_ap(k, m):
                off = 512 * k + 256 + 128 * m
                return cst[:, off:off + 128]

            def w2_ap(m):
                off = W2_OFF + m * 2
                return cst[:, off:off + 2]

            def i4_ap(b):
                off = I4_OFF + 4 * b
                return cst[0:1, off:off + 4]

            iota = cst[:, IOTA_OFF:IOTA_OFF + gw]

            def bm_ap(col):
                off = 3 + col
                return bf[:, off:off + 1]

            b1_ap = [bf[:, 0:1], bf[:, 1:2]]
            b2_ap = bf[0:1, 2:3]

            # ---- PE p-state warmup while first DMAs land ----
            warm = consts.tile([128, 64], f16, tag="warm")
            nc.gpsimd.memset(warm[:], 0.0)
            wps = rawpool.tile([64, 48], f32, tag="wps")
            for _ in range(44):
                nc.tensor.matmul(wps[:], warm[:, 0:64], warm[:, 0:48],
                                 start=True, stop=True)

            # seg rhs layout: [diff(256) | ones(2)] -> out cols 0:256 values,
            # 256:258 exp-sums
            seg = segpool.tile([gw, D + 2], f32, tag="seg")

            def nblocks_of(t):
                # trailing skip_blocks 128-blocks of the last tile are pad
                if t == n_tiles - 1:
                    return 4 - skip_blocks
                return 4

            def seg_mm(sw_f, dn_f, ti_f, t_f):
                nb = nblocks_of(t_f)
                for bb in range(nb):
                    nc.tensor.matmul(seg[:], sw_f[:, bb, :],
                                     dn_f[:, 4 * ti_f + bb, :],
                                     start=(t_f == 0 and bb == 0),
                                     stop=(t_f == n_tiles - 1 and bb == nb - 1),
                                     skip_group_check=True)

            fetch_dn(0)
            pending = []
            for g in range(n_groups):
                if g + 2 < n_groups:
                    fetch(g + 2)
                if g + 1 < n_groups:
                    fetch_dn(g + 1)
                e1, e2, dn = fetched[g]
                del fetched[g]

                # shared-bank PSUM pairs for raw ([2,512] rows at 32ti) and
                # ewT ([128,4] cols at 4ti)
                rawp = rawpool.tile([64, TILE_N], f32, tag="raw")
                ewt_ps = etpool.tile([128, 8], f32, tag="ewt_ps")

                def flush_one():
                    if not pending:
                        return
                    seg_mm(*pending.pop(0))

                last_g = (g == n_groups - 1)
                fill_g = (g >= n_groups - 2)

                # z^T [128, 512] per (tile, fo-chunk); each weight chunk is
                # loaded once and streamed against both tiles of the group
                zc = [[None, None], [None, None]]

                def mlp_t(ti):
                    for m in range(2):
                        zc[ti][m] = zpool.tile([128, TILE_N], f32, tag="zr",
                                               name=f"z_{g}_{ti}_{m}")
                        for wi, (wsel, esrc, k) in enumerate(
                                [(0, e1, 0), (1, e2, 0), (0, e1, 1), (1, e2, 1)]):
                            wmat = wa_ap(k, m) if wsel == 0 else wb_ap(k, m)
                            nc.tensor.matmul(
                                zc[ti][m][:], wmat,
                                esrc[:, k, bass.ts(ti, TILE_N)],
                                start=(wi == 0), stop=(wi == 3))

                def relu_t(ti):
                    h = hpool.tile([128, 2, TILE_N], f16, tag="h")
                    for m in range(2):
                        nc.scalar.activation(h[:, m, :], zc[ti][m][:], AF.Relu,
                                             bias=b1_ap[m], scale=1.0)
                    return h

                def raw_t(ti, h):
                    nc.tensor.matmul(rawp[32 * ti:32 * ti + 2, :], w2_ap(0),
                                     h[:, 0, :],
                                     start=True, stop=False,
                                     skip_group_check=True)
                    nc.tensor.matmul(rawp[32 * ti:32 * ti + 2, :], w2_ap(1),
                                     h[:, 1, :],
                                     start=False, stop=True,
                                     skip_group_check=True)

                def exp_t(ti):
                    ew = spool.tile([1, TILE_N], f16, tag="ew")
                    nc.scalar.activation(ew[:], rawp[32 * ti:32 * ti + 1, :],
                                         AF.Exp, bias=b2_ap, scale=1.0)
                    return ew

                if not last_g:
                    mlp_t(0)
                    mlp_t(1)
                    hs_ = [relu_t(0), relu_t(1)]
                    for ti in range(2):
                        raw_t(ti, hs_[ti])
                    ews = [exp_t(0), exp_t(1)]
                else:
                    # final group: per-tile chains interleaved with seg
                    # flushes so the ti0 softmax chain starts ~2us earlier
                    mlp_t(0)
                    h0 = relu_t(0)
                    flush_one()
                    raw_t(0, h0)
                    ew0 = exp_t(0)
                    mlp_t(1)
                    h1 = relu_t(1)
                    flush_one()
                    raw_t(1, h1)
                    flush_one()
                    ews = [ew0, exp_t(1)]

                # ewT [128, 4] at cols 4ti..: outer products with I4 rows,
                # all 8 back to back in the shared bank
                for ti in range(2):
                    for b in range(4):
                        nc.tensor.matmul(ewt_ps[:, 4 * ti:4 * ti + 4],
                                         ews[ti][:, bass.ts(b, 128)],
                                         i4_ap(b),
                                         start=(ti == 0 and b == 0),
                                         stop=(ti == 1 and b == 3),
                                         skip_group_check=True)
                if last_g:
                    flush_one()

                for ti in range(2):
                    t = 2 * g + ti
                    nb = nblocks_of(t)
                    ewt = spool.tile([128, 4], f32, tag="ewt")
                    nc.vector.tensor_copy(ewt[:], ewt_ps[:, 4 * ti:4 * ti + 4])

                    # Sw[:, b, :] = (iota == bm_col) * ewt_col  (Pool + DVE)
                    sw = spool.tile([128, 4, gw], f16, tag="sw")
                    for b in range(nb):
                        eng = nc.gpsimd if b == 0 else nc.vector
                        eng.tensor_scalar(
                            sw[:, b, :], iota, bm_ap(4 * t + b),
                            ewt[:, b:b + 1], op0=ALU.is_equal, op1=ALU.mult)

                    pending.append((sw, dn, ti, t))

                # segment accumulate, deferred by a full group so the Sw
                # chain has ~2 tiles of slack before the PE needs its output
                # (the last two groups defer everything so the final group's
                # latency chain is covered by old seg work)
                if not fill_g:
                    ready = [p for p in pending if p[3] < 2 * g]
                    pending = [p for p in pending if p[3] >= 2 * g]
                    for p in ready:
                        seg_mm(*p)

            while pending:
                seg_mm(*pending.pop(0))

            # tail: out = seg[:, 0:256] / max(seg[:, 256], eps)

            ssum = spool.tile([gw, 1], f32, tag="ssum")
            nc.vector.tensor_scalar_max(ssum[:], seg[:, D:D + 1], 1e-30)
            rec = spool.tile([gw, 1], f32, tag="rec")
            nc.vector.reciprocal(rec[:], ssum[:])
            ot = spool.tile([gw, D], f32, tag="ot")
            nc.vector.tensor_scalar_mul(ot[:], seg[:, 0:D], rec[:])
            hgw = gw // 2
            nc.sync.dma_start(out_d[0:hgw, :], ot[0:hgw, :])
            nc.sync.dma_start(out_d[hgw:gw, :], ot[hgw:gw, :])

    nc.compile()
    _CACHE[(cap, gw, skip_blocks)] = nc
    return nc


def _prepare(out_gnn, batch_input, W1, b1, W2, b2):
    out_gnn = np.asarray(out_gnn, dtype=np.float32)
    batch = np.asarray(batch_input, dtype=np.int64)
    W1 = np.asarray(W1, dtype=np.float32)
    b1 = np.asarray(b1, dtype=np.float32)
    W2 = np.asarray(W2, dtype=np.float32)
    b2 = np.asarray(b2, dtype=np.float32)

    half = out_gnn.shape[0] // 2
    batch = batch[:half]
    e1_all, e2_all = out_gnn[:half], out_gnn[half:]

    # Node-balanced, graph-aligned contiguous cuts. Core c handles graphs
    # [gcut[c], gcut[c+1]) and the matching contiguous node range.
    counts = np.bincount(batch, minlength=NUM_GRAPHS)
    ccum = np.concatenate([[0], np.cumsum(counts)])  # node offset per graph
    g_used = int(np.max(np.nonzero(counts)[0])) + 1 if counts.any() else 1
    gcut = np.zeros(N_CORES + 1, dtype=np.int64)
    gcut[N_CORES] = g_used
    for c in range(1, N_CORES):
        g = int(np.searchsorted(ccum, ccum[g_used] * c / N_CORES, side="left"))
        gcut[c] = min(max(g, gcut[c - 1]), g_used)
    spans = gcut[1:] - gcut[:-1]
    if spans.max() > 128:
        gcut = np.round(np.linspace(0, g_used, N_CORES + 1)).astype(np.int64)
        spans = gcut[1:] - gcut[:-1]
        if spans.max() > 128:
            raise ValueError(f"graph window {spans.max()} > 128 unsupported")

    nbounds = ccum[gcut]  # node boundaries per core
    gw = int(max(2, ((spans.max() + 1) // 2) * 2))
    max_n = int((nbounds[1:] - nbounds[:-1]).max())
    grp = 2 * TILE_N
    cap = max(grp, ((max_n + grp - 1) // grp) * grp)
    nblocks = cap // 128

    skip_blocks = min(3, int((cap - max_n) // 128))
    nc = _build_program(cap, gw, skip_blocks)

    # host-folded MLP weights (fp64 for exactness)
    W1a = W1[0:D].astype(np.float64)
    W1b = W1[D:2 * D].astype(np.float64)
    W1c = W1[2 * D:3 * D].astype(np.float64)
    WA = (W1a + W1c).astype(np.float32)
    WB = (W1b - W1c).astype(np.float32)

    def chunkw(w, k):  # [256,256] -> [m*128+n] fp16 cols on 128 rows, chunk k
        return np.ascontiguousarray(
            w[k * 128:(k + 1) * 128].astype(np.float16).reshape(128, 256))

    # fp16 const blob
    IOTA_OFF = 1044
    NC16 = IOTA_OFF + gw
    cst_common = np.zeros((128, NC16), dtype=np.float16)
    for k in range(2):
        cst_common[:, 512 * k:512 * k + 256] = chunkw(WA, k)
        cst_common[:, 512 * k + 256:512 * k + 512] = chunkw(WB, k)
    cst_common[:, 1024:1028] = np.concatenate(
        [W2.astype(np.float16).reshape(2, 128, 1).transpose(1, 0, 2),
         np.zeros((128, 2, 1), np.float16)], axis=2).reshape(128, 4)
    cst_common[0, 1028:1044] = np.eye(4, dtype=np.float16).reshape(16)
    cst_common[:, IOTA_OFF:IOTA_OFF + gw] = np.arange(gw, dtype=np.float16)

    bf_common = np.zeros((128, 3 + nblocks), dtype=np.float32)
    bf_common[:, 0] = b1[0:128]
    bf_common[:, 1] = b1[128:256]
    bf_common[0, 2] = b2[0]

    in_maps = []
    for c in range(N_CORES):
        s, e = int(nbounds[c]), int(nbounds[c + 1])
        n_c = e - s
        e1t = np.zeros((2, 128, cap), dtype=np.float16)
        e2t = np.zeros((2, 128, cap), dtype=np.float16)
        e1t[:, :, :n_c] = e1_all[s:e].astype(np.float16).T.reshape(2, 128, n_c)
        e2t[:, :, :n_c] = e2_all[s:e].astype(np.float16).T.reshape(2, 128, n_c)
        # node-major diff blob [128, nblocks, 258]; cols 256:258 = 1.0
        dnb = np.zeros((nblocks, 128, D + 2), dtype=np.float16)
        dpad = np.zeros((cap, D), dtype=np.float16)
        dpad[:n_c] = (e1_all[s:e] - e2_all[s:e]).astype(np.float16)
        dnb[:, :, 0:D] = dpad.reshape(nblocks, 128, D)
        dnb[:, :, D:D + 2] = 1.0
        dnb = np.ascontiguousarray(dnb.transpose(1, 0, 2))
        bmv = np.full(cap, 999.0, dtype=np.float32)
        bmv[:n_c] = (batch[s:e] - gcut[c]).astype(np.float32)
        bf = bf_common.copy()
        bf[:, 3:3 + nblocks] = bmv.reshape(nblocks, 128).T
        in_maps.append({
            "e1t": e1t, "e2t": e2t, "dn": dnb, "cst": cst_common, "bf": bf,
        })
    return nc, in_maps, gcut


def _enable_ldw_opt():
    """Re-enable the compiler's weight-load optimization (off by default in
    this container's flag set); harmless no-op if the flag isn't present."""
    try:
        from concourse.compiler_utils import get_compiler_flags, set_compiler_flags
        flags = [f.replace("--enable-ldw-opt=false", "--enable-ldw-opt=true")
                 for f in get_compiler_flags()]
        set_compiler_flags(flags)
    except Exception:
        pass


def kernel(out_gnn, batch_input, W1, b1, W2, b2):
    import concourse.bass_utils as bass_utils

    _enable_ldw_opt()
    nc, in_maps, gcut = _prepare(out_gnn, batch_input, W1, b1, W2, b2)

    trace_dir = os.environ.get("NODEATT_TRACE_DIR")
    kw = {}
    if trace_dir:
        kw = {"trace": True, "tmpdir": trace_dir}
    res = bass_utils.run_bass_kernel_spmd(
        nc, in_maps, core_ids=list(range(N_CORES)), **kw)
    if trace_dir:
        kernel.last_exec_time_ns = res.exec_time_ns
        kernel.last_results = res

    out = np.zeros((NUM_GRAPHS, D), dtype=np.float32)
    for c in range(N_CORES):
        span = int(gcut[c + 1] - gcut[c])
        if span > 0:
            out[gcut[c]:gcut[c + 1]] = res.results[c]["out"][:span]
    return out


# revision 36
# speedup vs baseline: 1.0304x; 1.0021x over previous
"""Trainium2 Bass kernel for nn_NodeAttDiff (segment-reduce node attention).

Math (reference):
    e1, e2 = out_gnn[:N], out_gnn[N:]          # N = 200000, D = 256
    diff   = e1 - e2
    h      = relu([e1 e2 diff] @ W1 + b1)      # folded: e1@WA + e2@WB, WA=W1a+W1c, WB=W1b-W1c
    raw    = (h @ W2 + b2)[:, 0]
    att    = segment_softmax(raw, batch)       # 512 contiguous segments (batch sorted)
    out    = segment_sum(att[:,None] * diff)   # [512, 256]

Device strategy (8 cores, graph-partitioned data parallel):
    - 64-ish graphs / core; each core gets its contiguous node slice (padded to
      a common capacity, pad nodes carry out-of-range segment id -> dropped).
    - Softmax max-subtraction is skipped (raw is O(5); exp is safe in fp32) and
      normalization is algebraic:  out_g = (sum_n w_n diff_n) / (sum_n w_n),
      w_n = exp(raw_n + b2)  -- no per-node att materialization.
    - Host pre-transposes e1/e2 to feature-major [2,128,cap] fp16 AND diff to
      node-major [128, nblocks, 258] fp16 (cols 256:258 = 1.0 -> the exp-sum
      denominator rides along as two extra columns of the segment matmul).
      Shipping diff node-major removes all on-device PE transposes.
    - Per group of 2x512-node tiles (weight chunks stream against both tiles):
        z^T   = WA.T @ e1T + WB.T @ e2T                  (8 matmuls/tile, PSUM)
        h^T   = relu(z^T + b1)                           (ACT, PSUM->SBUF)
        raw   = W2.T @ h^T                               (2 matmuls -> [2,512])
        ew    = exp(raw + b2)                            (ACT -> SBUF row)
        ewT   = I4-trick transpose of ew -> [128,4]      (4 tiny matmuls)
        Sw    = (iota == seg_id) * ewT                   (Pool+DVE tensor_scalar x4)
        seg  += Sw.T @ dn_block                          (PSUM accumulate, whole core)
      raw/ewT pairs of the two tiles share one PSUM bank each.
    - All constants ride in a few split DMAs (fp16 blob + fp32 blob) behind
      the first group's e-tensors; 44 dummy matmuls keep the PE busy while
      the first DMAs land so the DVFS p-state ramps before real work.
    - dn fetches trail e-fetches by a group (seg use is deferred a group),
      keeping head DMA bandwidth on the MLP-critical e-tiles.
    - Tail: out = seg[:,0:256] * recip(max(seg[:,256], eps)), DMA out [gw,256].
"""

import os
import numpy as np

NUM_GRAPHS = 512
N_CORES = 8
D = 256
TILE_N = 512  # nodes per tile


_CACHE = {}


def _build_program(cap: int, gw: int, skip_blocks: int = 0):
    """Build + compile the SPMD Bass program; `cap` nodes and a `gw`-graph
    window per core. The trailing `skip_blocks` 128-node blocks are pad on
    every core and get no sw/seg work."""
    if (cap, gw, skip_blocks) in _CACHE:
        return _CACHE[(cap, gw, skip_blocks)]

    from contextlib import ExitStack
    import concourse.bass as bass
    import concourse.tile as tile
    import concourse.bacc as bacc
    import concourse.mybir as mybir

    f32 = mybir.dt.float32
    f16 = mybir.dt.float16
    AF = mybir.ActivationFunctionType
    ALU = mybir.AluOpType

    assert cap % (2 * TILE_N) == 0
    n_tiles = cap // TILE_N
    n_groups = n_tiles // 2
    nblocks = cap // 128

    # fp16 const blob: per-k weight banks [waKm0 waKm1 wbKm0 wbKm1], k=0,1
    W2_OFF = 1024         # [m, c] flat m*2+c, 4 cols
    I4_OFF = 1028         # 16 cols (partition 0 only)
    IOTA_OFF = 1044       # gw cols
    NC16 = IOTA_OFF + gw
    # fp32 blob: b1 (2 cols), b2 (1 col), bm (nblocks cols)
    NC32 = 3 + nblocks

    nc = bacc.Bacc("TRN2", target_bir_lowering=False, debug=False,
                   num_devices=N_CORES)

    e1t_d = nc.dram_tensor("e1t", [2, 128, cap], f16, kind="ExternalInput").ap()
    e2t_d = nc.dram_tensor("e2t", [2, 128, cap], f16, kind="ExternalInput").ap()
    dn_d = nc.dram_tensor("dn", [128, nblocks, D + 2], f16,
                          kind="ExternalInput").ap()
    cst_d = nc.dram_tensor("cst", [128, NC16], f16, kind="ExternalInput").ap()
    bf_d = nc.dram_tensor("bf", [128, NC32], f32, kind="ExternalInput").ap()
    out_d = nc.dram_tensor("out", [gw, D], f32, kind="ExternalOutput").ap()

    with tile.TileContext(nc) as tc:
        with ExitStack() as ctx:
            consts = ctx.enter_context(tc.tile_pool(name="consts", bufs=1))
            epool = ctx.enter_context(tc.tile_pool(name="epool", bufs=6))
            dnpool = ctx.enter_context(tc.tile_pool(name="dnpool", bufs=4))
            hpool = ctx.enter_context(tc.tile_pool(name="hpool", bufs=4))
            spool = ctx.enter_context(tc.tile_pool(name="spool", bufs=8))
            zpool = ctx.enter_context(
                tc.tile_pool(name="zpool", bufs=4, space=bass.MemorySpace.PSUM))
            rawpool = ctx.enter_context(
                tc.tile_pool(name="rawpool", bufs=1, space=bass.MemorySpace.PSUM))
            etpool = ctx.enter_context(
                tc.tile_pool(name="etpool", bufs=1, space=bass.MemorySpace.PSUM))
            segpool = ctx.enter_context(
                tc.tile_pool(name="segpool", bufs=1, space=bass.MemorySpace.PSUM))

            # ---- group input fetch (k-split DMAs; first chunks preempt
            # the consts so the MLP can start as early as possible) ----
            fetched = {}

            def fetch(g, tail=True):
                e1 = epool.tile([128, 2, 2 * TILE_N], f16, tag="e1")
                e2 = epool.tile([128, 2, 2 * TILE_N], f16, tag="e2")
                dn = dnpool.tile([128, 8, D + 2], f16, tag="dn")
                gsl = bass.ts(g, 2 * TILE_N)
                nc.sync.dma_start(e1[:, 0, :], e1t_d[0, :, gsl])
                nc.sync.dma_start(e2[:, 0, :], e2t_d[0, :, gsl])
                fetched[g] = (e1, e2, dn)
                if tail:
                    fetch_tail(g)

            def fetch_tail(g):
                e1, e2, dn = fetched[g]
                gsl = bass.ts(g, 2 * TILE_N)
                nc.sync.dma_start(e1[:, 1, :], e1t_d[1, :, gsl])
                nc.sync.dma_start(e2[:, 1, :], e2t_d[1, :, gsl])

            def fetch_dn(g):
                dn = fetched[g][2]
                nc.sync.dma_start(dn[:], dn_d[:, bass.ts(g, 8), :])

            # group 0, finest granularity: the first MLP matmuls need
            # e1/e2 k=0 tile-0 halves plus the k=0 weight chunks only
            e1_0 = epool.tile([128, 2, 2 * TILE_N], f16, tag="e1")
            e2_0 = epool.tile([128, 2, 2 * TILE_N], f16, tag="e2")
            dn_0 = dnpool.tile([128, 8, D + 2], f16, tag="dn")
            fetched[0] = (e1_0, e2_0, dn_0)
            cst = consts.tile([128, NC16], f16, tag="cst")
            bf = consts.tile([128, NC32], f32, tag="bf")
            # issue order follows the ti-outer MLP's consumption order:
            # tile-0 needs k0 AND k1 (data + weights) within 4 matmuls
            hs = bass.ts(0, TILE_N)
            t1 = bass.ts(1, TILE_N)
            nc.sync.dma_start(e1_0[:, 0, 0:TILE_N], e1t_d[0, :, hs])
            nc.sync.dma_start(e2_0[:, 0, 0:TILE_N], e2t_d[0, :, hs])
            nc.sync.dma_start(cst[:, 0:512], cst_d[:, 0:512])        # k0 weights
            nc.sync.dma_start(e1_0[:, 1, 0:TILE_N], e1t_d[1, :, hs])
            nc.sync.dma_start(e2_0[:, 1, 0:TILE_N], e2t_d[1, :, hs])
            nc.sync.dma_start(cst[:, 512:1024], cst_d[:, 512:1024])  # k1 weights
            nc.sync.dma_start(bf[:], bf_d[:])
            nc.sync.dma_start(e1_0[:, 0, TILE_N:], e1t_d[0, :, t1])
            nc.sync.dma_start(e2_0[:, 0, TILE_N:], e2t_d[0, :, t1])
            nc.sync.dma_start(e1_0[:, 1, TILE_N:], e1t_d[1, :, t1])
            nc.sync.dma_start(e2_0[:, 1, TILE_N:], e2t_d[1, :, t1])
            nc.sync.dma_start(cst[:, 1024:], cst_d[:, 1024:])        # w2/i4/iota
            if 1 < n_groups:
                fetch(1)

            def wa_ap(k, m):
                off = 512 * k + 128 * m
                return cst[:, off:off + 128]

            def wb_ap(k, m):
                off = 512 * k + 256 + 128 * m
                return cst[:, off:off + 128]

            def w2_ap(m):
                off = W2_OFF + m * 2
                return cst[:, off:off + 2]

            def i4_ap(b):
                off = I4_OFF + 4 * b
                return cst[0:1, off:off + 4]

            iota = cst[:, IOTA_OFF:IOTA_OFF + gw]

            def bm_ap(col):
                off = 3 + col
                return bf[:, off:off + 1]

            b1_ap = [bf[:, 0:1], bf[:, 1:2]]
            b2_ap = bf[0:1, 2:3]

            # ---- PE p-state warmup while first DMAs land ----
            warm = consts.tile([128, 64], f16, tag="warm")
            nc.gpsimd.memset(warm[:], 0.0)
            wps = rawpool.tile([64, 48], f32, tag="wps")
            for _ in range(44):
                nc.tensor.matmul(wps[:], warm[:, 0:64], warm[:, 0:48],
                                 start=True, stop=True)

            # seg rhs layout: [diff(256) | ones(2)] -> out cols 0:256 values,
            # 256:258 exp-sums
            seg = segpool.tile([gw, D + 2], f32, tag="seg")

            def nblocks_of(t):
                # trailing skip_blocks 128-blocks of the last tile are pad
                if t == n_tiles - 1:
                    return 4 - skip_blocks
                return 4

            def seg_mm(sw_f, dn_f, ti_f, t_f):
                nb = nblocks_of(t_f)
                for bb in range(nb):
                    nc.tensor.matmul(seg[:], sw_f[:, bb, :],
                                     dn_f[:, 4 * ti_f + bb, :],
                                     start=(t_f == 0 and bb == 0),
                                     stop=(t_f == n_tiles - 1 and bb == nb - 1),
                                     skip_group_check=True)

            fetch_dn(0)
            pending = []
            for g in range(n_groups):
                if g + 2 < n_groups:
                    fetch(g + 2)
                if g + 1 < n_groups:
                    fetch_dn(g + 1)
                e1, e2, dn = fetched[g]
                del fetched[g]

                # shared-bank PSUM pairs for raw ([2,512] rows at 32ti) and
                # ewT ([128,4] cols at 4ti)
                rawp = rawpool.tile([64, TILE_N], f32, tag="raw")
                ewt_ps = etpool.tile([128, 8], f32, tag="ewt_ps")

                def flush_one():
                    if not pending:
                        return
                    seg_mm(*pending.pop(0))

                last_g = (g == n_groups - 1)
                fill_g = (g >= n_groups - 3)

                # z^T [128, 512] per (tile, fo-chunk); each weight chunk is
                # loaded once and streamed against both tiles of the group
                zc = [[None, None], [None, None]]

                def mlp_t(ti):
                    for m in range(2):
                        zc[ti][m] = zpool.tile([128, TILE_N], f32, tag="zr",
                                               name=f"z_{g}_{ti}_{m}")
                        for wi, (wsel, esrc, k) in enumerate(
                                [(0, e1, 0), (1, e2, 0), (0, e1, 1), (1, e2, 1)]):
                            wmat = wa_ap(k, m) if wsel == 0 else wb_ap(k, m)
                            nc.tensor.matmul(
                                zc[ti][m][:], wmat,
                                esrc[:, k, bass.ts(ti, TILE_N)],
                                start=(wi == 0), stop=(wi == 3))

                def relu_t(ti):
                    h = hpool.tile([128, 2, TILE_N], f16, tag="h")
                    for m in range(2):
                        nc.scalar.activation(h[:, m, :], zc[ti][m][:], AF.Relu,
                                             bias=b1_ap[m], scale=1.0)
                    return h

                def raw_t(ti, h):
                    nc.tensor.matmul(rawp[32 * ti:32 * ti + 2, :], w2_ap(0),
                                     h[:, 0, :],
                                     start=True, stop=False,
                                     skip_group_check=True)
                    nc.tensor.matmul(rawp[32 * ti:32 * ti + 2, :], w2_ap(1),
                                     h[:, 1, :],
                                     start=False, stop=True,
                                     skip_group_check=True)

                def exp_t(ti):
                    ew = spool.tile([1, TILE_N], f16, tag="ew")
                    nc.scalar.activation(ew[:], rawp[32 * ti:32 * ti + 1, :],
                                         AF.Exp, bias=b2_ap, scale=1.0)
                    return ew

                if not last_g:
                    mlp_t(0)
                    mlp_t(1)
                    hs_ = [relu_t(0), relu_t(1)]
                    for ti in range(2):
                        raw_t(ti, hs_[ti])
                    ews = [exp_t(0), exp_t(1)]
                else:
                    # final group: per-tile chains interleaved with seg
                    # flushes so the ti0 softmax chain starts ~2us earlier
                    mlp_t(0)
                    h0 = relu_t(0)
                    flush_one()
                    raw_t(0, h0)
                    ew0 = exp_t(0)
                    mlp_t(1)
                    h1 = relu_t(1)
                    flush_one()
                    raw_t(1, h1)
                    flush_one()
                    ews = [ew0, exp_t(1)]

                # ewT [128, 4] at cols 4ti..: outer products with I4 rows,
                # all 8 back to back in the shared bank
                for ti in range(2):
                    for b in range(4):
                        nc.tensor.matmul(ewt_ps[:, 4 * ti:4 * ti + 4],
                                         ews[ti][:, bass.ts(b, 128)],
                                         i4_ap(b),
                                         start=(ti == 0 and b == 0),
                                         stop=(ti == 1 and b == 3),
                                         skip_group_check=True)
                if last_g:
                    flush_one()

                for ti in range(2):
                    t = 2 * g + ti
                    nb = nblocks_of(t)
                    ewt = spool.tile([128, 4], f32, tag="ewt")
                    nc.scalar.activation(ewt[:], ewt_ps[:, 4 * ti:4 * ti + 4],
                                         AF.Copy, bias=0.0, scale=1.0)

                    # Sw[:, b, :] = (iota == bm_col) * ewt_col  (Pool + DVE)
                    sw = spool.tile([128, 4, gw], f16, tag="sw")
                    for b in range(nb):
                        eng = nc.gpsimd if b == 0 else nc.vector
                        eng.tensor_scalar(
                            sw[:, b, :], iota, bm_ap(4 * t + b),
                            ewt[:, b:b + 1], op0=ALU.is_equal, op1=ALU.mult)

                    pending.append((sw, dn, ti, t))

                # segment accumulate, deferred by a full group so the Sw
                # chain has ~2 tiles of slack before the PE needs its output
                # (the last two groups defer everything so the final group's
                # latency chain is covered by old seg work)
                if not fill_g:
                    ready = [p for p in pending if p[3] < 2 * g]
                    pending = [p for p in pending if p[3] >= 2 * g]
                    for p in ready:
                        seg_mm(*p)

            while pending:
                seg_mm(*pending.pop(0))

            # tail: out = seg[:, 0:256] / max(seg[:, 256], eps)

            ssum = spool.tile([gw, 1], f32, tag="ssum")
            nc.vector.tensor_scalar_max(ssum[:], seg[:, D:D + 1], 1e-30)
            rec = spool.tile([gw, 1], f32, tag="rec")
            nc.vector.reciprocal(rec[:], ssum[:])
            ot = spool.tile([gw, D], f32, tag="ot")
            nc.vector.tensor_scalar_mul(ot[:], seg[:, 0:D], rec[:])
            hgw = gw // 2
            nc.sync.dma_start(out_d[0:hgw, :], ot[0:hgw, :])
            nc.sync.dma_start(out_d[hgw:gw, :], ot[hgw:gw, :])

    nc.compile()
    _CACHE[(cap, gw, skip_blocks)] = nc
    return nc


def _prepare(out_gnn, batch_input, W1, b1, W2, b2):
    out_gnn = np.asarray(out_gnn, dtype=np.float32)
    batch = np.asarray(batch_input, dtype=np.int64)
    W1 = np.asarray(W1, dtype=np.float32)
    b1 = np.asarray(b1, dtype=np.float32)
    W2 = np.asarray(W2, dtype=np.float32)
    b2 = np.asarray(b2, dtype=np.float32)

    half = out_gnn.shape[0] // 2
    batch = batch[:half]
    e1_all, e2_all = out_gnn[:half], out_gnn[half:]

    # Node-balanced, graph-aligned contiguous cuts. Core c handles graphs
    # [gcut[c], gcut[c+1]) and the matching contiguous node range.
    counts = np.bincount(batch, minlength=NUM_GRAPHS)
    ccum = np.concatenate([[0], np.cumsum(counts)])  # node offset per graph
    g_used = int(np.max(np.nonzero(counts)[0])) + 1 if counts.any() else 1
    gcut = np.zeros(N_CORES + 1, dtype=np.int64)
    gcut[N_CORES] = g_used
    for c in range(1, N_CORES):
        g = int(np.searchsorted(ccum, ccum[g_used] * c / N_CORES, side="left"))
        gcut[c] = min(max(g, gcut[c - 1]), g_used)
    spans = gcut[1:] - gcut[:-1]
    if spans.max() > 128:
        gcut = np.round(np.linspace(0, g_used, N_CORES + 1)).astype(np.int64)
        spans = gcut[1:] - gcut[:-1]
        if spans.max() > 128:
            raise ValueError(f"graph window {spans.max()} > 128 unsupported")

    nbounds = ccum[gcut]  # node boundaries per core
    gw = int(max(2, ((spans.max() + 1) // 2) * 2))
    max_n = int((nbounds[1:] - nbounds[:-1]).max())
    grp = 2 * TILE_N
    cap = max(grp, ((max_n + grp - 1) // grp) * grp)
    nblocks = cap // 128

    skip_blocks = min(3, int((cap - max_n) // 128))
    nc = _build_program(cap, gw, skip_blocks)

    # host-folded MLP weights (fp64 for exactness)
    W1a = W1[0:D].astype(np.float64)
    W1b = W1[D:2 * D].astype(np.float64)
    W1c = W1[2 * D:3 * D].astype(np.float64)
    WA = (W1a + W1c).astype(np.float32)
    WB = (W1b - W1c).astype(np.float32)

    def chunkw(w, k):  # [256,256] -> [m*128+n] fp16 cols on 128 rows, chunk k
        return np.ascontiguousarray(
            w[k * 128:(k + 1) * 128].astype(np.float16).reshape(128, 256))

    # fp16 const blob
    IOTA_OFF = 1044
    NC16 = IOTA_OFF + gw
    cst_common = np.zeros((128, NC16), dtype=np.float16)
    for k in range(2):
        cst_common[:, 512 * k:512 * k + 256] = chunkw(WA, k)
        cst_common[:, 512 * k + 256:512 * k + 512] = chunkw(WB, k)
    cst_common[:, 1024:1028] = np.concatenate(
        [W2.astype(np.float16).reshape(2, 128, 1).transpose(1, 0, 2),
         np.zeros((128, 2, 1), np.float16)], axis=2).reshape(128, 4)
    cst_common[0, 1028:1044] = np.eye(4, dtype=np.float16).reshape(16)
    cst_common[:, IOTA_OFF:IOTA_OFF + gw] = np.arange(gw, dtype=np.float16)

    bf_common = np.zeros((128, 3 + nblocks), dtype=np.float32)
    bf_common[:, 0] = b1[0:128]
    bf_common[:, 1] = b1[128:256]
    bf_common[0, 2] = b2[0]

    in_maps = []
    for c in range(N_CORES):
        s, e = int(nbounds[c]), int(nbounds[c + 1])
        n_c = e - s
        e1t = np.zeros((2, 128, cap), dtype=np.float16)
        e2t = np.zeros((2, 128, cap), dtype=np.float16)
        e1t[:, :, :n_c] = e1_all[s:e].astype(np.float16).T.reshape(2, 128, n_c)
        e2t[:, :, :n_c] = e2_all[s:e].astype(np.float16).T.reshape(2, 128, n_c)
        # node-major diff blob [128, nblocks, 258]; cols 256:258 = 1.0
        dnb = np.zeros((nblocks, 128, D + 2), dtype=np.float16)
        dpad = np.zeros((cap, D), dtype=np.float16)
        dpad[:n_c] = (e1_all[s:e] - e2_all[s:e]).astype(np.float16)
        dnb[:, :, 0:D] = dpad.reshape(nblocks, 128, D)
        dnb[:, :, D:D + 2] = 1.0
        dnb = np.ascontiguousarray(dnb.transpose(1, 0, 2))
        bmv = np.full(cap, 999.0, dtype=np.float32)
        bmv[:n_c] = (batch[s:e] - gcut[c]).astype(np.float32)
        bf = bf_common.copy()
        bf[:, 3:3 + nblocks] = bmv.reshape(nblocks, 128).T
        in_maps.append({
            "e1t": e1t, "e2t": e2t, "dn": dnb, "cst": cst_common, "bf": bf,
        })
    return nc, in_maps, gcut


def _enable_ldw_opt():
    """Re-enable the compiler's weight-load optimization (off by default in
    this container's flag set); harmless no-op if the flag isn't present."""
    try:
        from concourse.compiler_utils import get_compiler_flags, set_compiler_flags
        flags = [f.replace("--enable-ldw-opt=false", "--enable-ldw-opt=true")
                 for f in get_compiler_flags()]
        set_compiler_flags(flags)
    except Exception:
        pass


def kernel(out_gnn, batch_input, W1, b1, W2, b2):
    import concourse.bass_utils as bass_utils

    _enable_ldw_opt()
    nc, in_maps, gcut = _prepare(out_gnn, batch_input, W1, b1, W2, b2)

    trace_dir = os.environ.get("NODEATT_TRACE_DIR")
    kw = {}
    if trace_dir:
        kw = {"trace": True, "tmpdir": trace_dir}
    res = bass_utils.run_bass_kernel_spmd(
        nc, in_maps, core_ids=list(range(N_CORES)), **kw)
    if trace_dir:
        kernel.last_exec_time_ns = res.exec_time_ns
        kernel.last_results = res

    out = np.zeros((NUM_GRAPHS, D), dtype=np.float32)
    for c in range(N_CORES):
        span = int(gcut[c + 1] - gcut[c])
        if span > 0:
            out[gcut[c]:gcut[c + 1]] = res.results[c]["out"][:span]
    return out
